# revision 30
# baseline (speedup 1.0000x reference)
"""Trainium2 Bass kernel for nn_ExpandingLinear.

Reference computation (B=8192, F0=2048, E1=E2=256, O=1024, F2=2560):
    h1 = concat([x, relu(x[:, e1_parent] * e1_w)], 1)          # [B, 2304]
    h2 = concat([h1, relu(h1[:, e2_parent] * e2_w)], 1)        # [B, 2560]
    W  = scatter_add(zeros(O, F2), (w_rows, w_cols), w_vals)
    b  = scatter_add(zeros(O,), b_idx, b_vals)
    out = h2 @ W.T + b                                          # [B, O]

Algebraic reduction done on the host (weights only):
    relu(x * w) == w * relu(sign(w) * x) for scalar w, so every embed output
    column is (nonneg scalar) * relu(s * x[:, c]) for some source column c and
    sign s.  Folding each embed column's contribution through W gives

        out = x @ W0t + relu(S ⊙ xg) @ A + 1·bias

    where W0t = W[:, :2048].T, xg = the <=511 distinct (c, s) source columns,
    A is a small host-folded matrix, and the all-ones lhsT row adds the bias.

Device kernel (SPMD over 8 cores, batch-sharded 1024 rows/core):
    - DMA x_shard.T (16 k-tiles, scalar queue) + folded weights in n-halves
      ((16+RT) k-tiles, sync queue) + gathered sign-relu columns (RT tiles)
    - memset-sourced PE warm-up bridges the framework preamble to first data
    - ACT: rt = relu(sign * xg); last row is the all-ones bias row
    - PE: out[m,n] accumulates over k-tiles in 4 waves x 2 PSUM half-passes
      (half 0 k-major in DMA arrival order, half 1 group-major so DVE bank
      drains keep ahead of bank-recycling start-matmuls)
    - DVE: PSUM -> SBUF accumulate; last wave stores in 256-col chunks
"""

import numpy as np

import concourse.bass as bass
import concourse.tile as tile
from concourse import bacc, mybir
from concourse.bass_utils import run_bass_kernel_spmd

B, F0, E1, E2, O = 8192, 2048, 256, 256, 1024
F1 = F0 + E1
F2 = F1 + E2
N_CORES = 8
BS = B // N_CORES          # 1024 batch rows per core
P = 128                    # partitions
KT_X = F0 // P             # 16 k-tiles of raw x
N_HALF = 512               # matmul moving free dim (fp32 max)

# matmul operand dtype:
#   float32  — exact, but the PE runs fp32 at 4 cycles/row (~296 us)
#   float32r — TF32 datapath, 1 cycle/row, rel err ~3e-4 (~97 us)
#   bfloat16 — 1 cycle/row + half the DMA bytes, rel err ~2e-3  <== default
MATMUL_DT = mybir.dt.bfloat16

_CACHE = {}


def _fold_weights(e1_w, e2_w, w_vals, b_vals, e1_parent, e2_parent,
                  w_rows, w_cols, b_idx):
    """Host-side weight preprocessing: densify W/b and fold the two embed
    layers' contributions into (cols, signs, A) so the device computes
    out = x @ W0t + relu(sign*x[:, cols]) @ A + bias."""
    W = np.bincount(w_rows.astype(np.int64) * F2 + w_cols.astype(np.int64),
                    weights=w_vals.astype(np.float64),
                    minlength=O * F2).reshape(O, F2)
    bias = np.bincount(b_idx.astype(np.int64), weights=b_vals.astype(np.float64),
                       minlength=O)
    W0t = W[:, :F0].T          # [2048, 1024]
    W1t = W[:, F0:F1].T        # [256, 1024]  layer-1 embed rows
    W2t = W[:, F1:F2].T        # [256, 1024]  layer-2 embed rows

    # each embed column j contributes scale*relu(s*x[:, c]) with weight row w
    # accumulate per (c, s): A_map[(c, s)] += scale * w_row
    A_map = {}

    def acc(c, s, scale, wrow):
        if scale == 0.0:
            return
        key = (int(c), int(s))
        if key in A_map:
            A_map[key] = A_map[key] + scale * wrow
        else:
            A_map[key] = scale * wrow

    e1_parent = e1_parent.astype(np.int64)
    e2_parent = e2_parent.astype(np.int64)
    e1_w64 = e1_w.astype(np.float64)
    e2_w64 = e2_w.astype(np.float64)

    for j in range(E1):
        w = e1_w64[j]
        s = 1 if w >= 0 else -1
        acc(e1_parent[j], s, abs(w), W1t[j])
    for j in range(E2):
        q = e2_parent[j]
        w = e2_w64[j]
        if q < F0:
            s = 1 if w >= 0 else -1
            acc(q, s, abs(w), W2t[j])
        else:
            # refers to layer-1 embed column m1: h1e[:, m1] >= 0 always
            if w < 0:
                continue  # relu(negative * nonneg) == 0
            m1 = q - F0
            w1 = e1_w64[m1]
            s = 1 if w1 >= 0 else -1
            acc(e1_parent[m1], s, w * abs(w1), W2t[j])

    pairs = sorted(A_map.keys())
    n_pairs = len(pairs)
    # relu-block k-tiles; last row of the block is reserved for the bias row
    RT = max(1, -(-(n_pairs + 1) // P))
    n_rows = RT * P
    cols = np.zeros(n_rows, dtype=np.int64)
    signs = np.ones(n_rows, dtype=np.float32)
    A = np.zeros((n_rows, O), dtype=np.float64)
    for i, (c, s) in enumerate(pairs):
        cols[i] = c
        signs[i] = s
        A[i] = A_map[(c, s)]
    return (W0t.astype(np.float32), A.astype(np.float32),
            bias.astype(np.float32), cols, signs, RT)


def _build_program(RT):
    """Build + compile the SPMD Bass program (same for every core)."""
    KT = KT_X + RT  # total k-tiles
    MDT = MATMUL_DT
    nc = bacc.Bacc("TRN2", target_bir_lowering=False, debug=False,
                   num_devices=N_CORES)

    # fp32r relu sources stay f32 (the DVE write rounds); bf16 arrives bf16
    GDT = MDT if MDT == mybir.dt.bfloat16 else mybir.dt.float32
    xt_d = nc.dram_tensor("xt", [KT_X, P, BS], MDT, kind="ExternalInput")
    xg_d = nc.dram_tensor("xg", [RT, P, BS], GDT, kind="ExternalInput")
    wc_d = nc.dram_tensor("wc", [KT, P, O], MDT, kind="ExternalInput")
    sg_d = nc.dram_tensor("sg", [P, RT], mybir.dt.float32,
                          kind="ExternalInput")
    # [m, n, p, c] layout: each [128, 512] half-store is contiguous
    out_d = nc.dram_tensor("out", [BS // P, O // N_HALF, P, N_HALF],
                           mybir.dt.float32, kind="ExternalOutput")

    with tile.TileContext(nc) as tc:
        with (
            tc.tile_pool(name="sbuf", bufs=1) as pool,
            tc.tile_pool(name="outp", bufs=1) as outp,
            tc.tile_pool(name="psum", bufs=8, space="PSUM") as psum,
        ):
            sg_sb = pool.tile([P, RT], mybir.dt.float32, tag="sg")
            nc.gpsimd.dma_start(sg_sb[:], sg_d[:])

            # PE warm-up: data-independent matmuls on a memset constant tile
            # (no DMA dependency, so they start the moment the framework
            # preamble ends) flip the HAM clock gate to 2.4 GHz before the
            # real stream starts (cold matmuls run at 1.2 GHz)
            cst = pool.tile([P, 256], MDT, tag="cst", name="cst")
            nc.vector.memset(cst[:], 1.0)
            wps = psum.tile([P, N_HALF], mybir.dt.float32, tag="ps",
                            name="wps")
            for _ in range(10):
                nc.tensor.matmul(wps[:, :256], cst[:, :P], cst[:],
                                 start=True, stop=True)

            # lhsT k-tiles (16 raw x + RT sign-relu) and weight k-tiles
            lh = [pool.tile([P, BS], MDT, tag=f"x{kt}", name=f"x{kt}")
                  for kt in range(KT_X)]
            wc = [pool.tile([P, O], MDT, tag=f"w{kt}", name=f"w{kt}")
                  for kt in range(KT)]
            # first k-tile arrives in half chunks so the PE can start on
            # (m0, n0) after ~256 KiB instead of ~512 KiB
            H = BS // 2
            H2 = O // 2
            g_sbs = [pool.tile([P, BS], GDT, tag="g", name=f"g{t}",
                               bufs=RT) for t in range(RT)]
            # wc tiles go on sync in n-halves, lh tiles on scalar: two
            # issue queues, each with a 4-deep flow-control window that
            # paces a single queue to ~165 GB/s -- the head of the kernel
            # is delivery-bound, and 128 KiB wc halves advance the sync
            # window ~2x faster than full tiles.  xg rides after k3 (it
            # feeds the relu whose output isn't consumed until wave 4).
            for kt in range(KT):
                if kt == 0:
                    nc.sync.dma_start(wc[kt][:, :H2], wc_d[kt][:, :H2])
                    nc.scalar.dma_start(lh[kt][:, :H], xt_d[kt][:, :H])
                    nc.sync.dma_start(wc[kt][:, H2:], wc_d[kt][:, H2:])
                    nc.scalar.dma_start(lh[kt][:, H:], xt_d[kt][:, H:])
                else:
                    nc.sync.dma_start(wc[kt][:, :H2], wc_d[kt][:, :H2])
                    nc.sync.dma_start(wc[kt][:, H2:], wc_d[kt][:, H2:])
                    if kt < KT_X:
                        nc.scalar.dma_start(lh[kt][:], xt_d[kt])
                if kt == 3:
                    for t in range(RT):
                        eng = nc.sync if t < RT // 2 else nc.scalar
                        eng.dma_start(g_sbs[t][:], xg_d[t])
            # sign-relu of the gathered columns.  Emitted between wave 1
            # and wave 2 so its scheduler priority sits behind the wave-1
            # drains; the r tiles are only consumed by wave 4 (k >= 16).
            def emit_relu():
                for t in range(RT):
                    r_sb = pool.tile([P, BS], MDT, tag=f"r{t}",
                                     name=f"r{t}")
                    # bias row: xg's last row is all-ones with sign +1, so
                    # the sign-relu passes it through unchanged.  Runs on
                    # the ACT engine (out = relu(in * scale)): the DVE's
                    # in-order queue must stay free for PSUM bank drains --
                    # a relu there blocks on the trickling xg transfers and
                    # stalls the PE's bank-recycling matmuls for ~5 us.
                    nc.scalar.activation(r_sb[:], g_sbs[t][:],
                                         mybir.ActivationFunctionType.Relu,
                                         scale=sg_sb[:, t:t + 1])
                    lh.append(r_sb)

            # K-outer waves of KC tiles: PE consumes k-tiles in DMA arrival
            # order and never waits on far-away tiles. 16 (m, n) output
            # groups > 8 PSUM banks, so each wave runs two passes of 8
            # groups (second pass re-reads the same resident k-tiles).
            # Waves accumulate into o_sb via DVE.
            MT = BS // P           # 8 m-tiles
            NT = O // N_HALF       # 2 n-halves
            groups = [(m, n) for m in range(MT) for n in range(NT)]
            o_sbs = [outp.tile([P, O], mybir.dt.float32, tag=f"o{m}",
                               name=f"o{m}") for m in range(MT)]
            # small waves while the k-stream is in flight, one big wave
            # once everything is resident (fewer DVE accumulate ops)
            waves = [(0, 4), (4, 8), (8, 12), (12, KT)]
            for wi, (k0, k1) in enumerate(waves):
                if wi == 1:
                    emit_relu()
                last_wave = wi == len(waves) - 1
                for half in range(2):
                    gsl = groups[half * 8:(half + 1) * 8]
                    pss = {g: psum.tile([P, N_HALF], mybir.dt.float32,
                                        tag="ps", name="ps") for g in gsl}
                    if last_wave or half == 1:
                        # group-major: each group's start-matmul is followed
                        # by its own k-run (~0.86us) before the next group
                        # needs its recycled PSUM bank, so the 0.68us DVE
                        # drain of that bank always finishes in time.  A
                        # k-major second half would issue 8 bank-reusing
                        # start-matmuls 216ns apart against a serialized
                        # drain chain and stall the PE for several us.
                        # (Half 1 re-reads tiles half 0 already used, so
                        # DMA arrival order doesn't matter here.)
                        order = [(kt, g) for g in gsl for kt in range(k0, k1)]
                    else:
                        # k-major: consume k-tiles in DMA arrival order.
                        # n-outer within each k-tile: the n0 half of each
                        # wc tile lands ~0.8us before the n1 half, so the
                        # four n0 matmuls run while n1 is still in flight
                        gs = sorted(gsl, key=lambda g: (g[1], g[0]))
                        order = [(kt, g) for kt in range(k0, k1) for g in gs]
                    for kt, (m, n) in order:
                        nc.tensor.matmul(
                            pss[(m, n)][:],
                            lh[kt][:, m * P:(m + 1) * P],
                            wc[kt][:, n * N_HALF:(n + 1) * N_HALF],
                            start=(kt == k0),
                            stop=(kt == k1 - 1),
                        )
                        if kt != k1 - 1:
                            continue
                        # group complete: drain its PSUM bank immediately so
                        # the next half's matmuls find a free bank without
                        # waiting (DVE accumulate into the persistent o_sb)
                        if not last_wave:
                            osl = o_sbs[m][:, n * N_HALF:(n + 1) * N_HALF]
                            if wi == 0:
                                nc.vector.tensor_copy(osl, pss[(m, n)][:])
                            else:
                                nc.vector.tensor_add(osl, osl, pss[(m, n)][:])
                            continue
                        # last wave: drain + store in 256-col chunks so the
                        # final group's store chain after its last matmul is
                        # drain(0.35us) -> issue -> small transfer instead of
                        # a serialized 0.7us drain + 0.7us full-tile store
                        CH = 256
                        for ci in range(N_HALF // CH):
                            c0 = n * N_HALF + ci * CH
                            osl = o_sbs[m][:, c0:c0 + CH]
                            nc.vector.tensor_add(
                                osl, osl,
                                pss[(m, n)][:, ci * CH:(ci + 1) * CH])
                            # sync/scalar only: both are idle by now and
                            # their dma_start executes ~0.1us faster than
                            # gpsimd's, which matters on the final chain
                            eng = nc.sync if ci == 0 else nc.scalar
                            eng.dma_start(
                                out_d[m][n][:, ci * CH:(ci + 1) * CH], osl)

    nc.compile()
    return nc


def _round_tf32(a):
    """Round-to-nearest-even fp32 -> tf32 (10-bit mantissa), like the PE's
    fp32r datapath expects (low 13 mantissa bits zero)."""
    u = a.astype(np.float32).view(np.uint32)
    rb = (u >> np.uint32(13)) & np.uint32(1)
    u = (u + np.uint32(0x0FFF) + rb) & np.uint32(0xFFFFE000)
    return u.view(np.float32)


def kernel(input, e1_w, e2_w, w_vals, b_vals, e1_parent, e2_parent,
           w_rows, w_cols, b_idx):
    input = np.asarray(input, dtype=np.float32)
    W0t, A, bias, cols, signs, RT = _fold_weights(
        np.asarray(e1_w), np.asarray(e2_w), np.asarray(w_vals),
        np.asarray(b_vals), np.asarray(e1_parent), np.asarray(e2_parent),
        np.asarray(w_rows), np.asarray(w_cols), np.asarray(b_idx))

    KT = KT_X + RT
    # weight slab: [KT*128, O] = [W0t ; A-with-bias-row]
    wc = np.concatenate([W0t, A], axis=0)
    wc[KT * P - 1, :] = bias           # lhsT row is all-ones -> adds bias
    wc = np.ascontiguousarray(wc.reshape(KT, P, O), dtype=np.float32)
    sg = np.ascontiguousarray(signs.reshape(RT, P).T, dtype=np.float32)

    key = (RT, MATMUL_DT)
    if key not in _CACHE:
        _CACHE[key] = _build_program(RT)
    nc = _CACHE[key]

    xg_full = input[:, cols]           # [B, RT*128] gathered source columns
    xg_full[:, RT * P - 1] = 1.0       # all-ones bias column (sign is +1)
    xmm = input
    if MATMUL_DT == mybir.dt.float32r:
        xmm = _round_tf32(input)
        xg_full = _round_tf32(xg_full)  # relu/sign-mult commute with rounding
        wc = _round_tf32(wc)
    elif MATMUL_DT == mybir.dt.bfloat16:
        import ml_dtypes
        bf = np.dtype(ml_dtypes.bfloat16)
        xmm = input.astype(bf)
        xg_full = xg_full.astype(bf)
        wc = wc.astype(bf)
    in_maps = []
    for c in range(N_CORES):
        sl = slice(c * BS, (c + 1) * BS)
        xt_c = np.ascontiguousarray(xmm[sl].T.reshape(KT_X, P, BS))
        xg_c = np.ascontiguousarray(xg_full[sl].T.reshape(RT, P, BS))
        in_maps.append({"xt": xt_c, "xg": xg_c, "wc": wc, "sg": sg})

    res = run_bass_kernel_spmd(nc, in_maps, list(range(N_CORES)))
    out = np.concatenate(
        [res.results[c]["out"].transpose(0, 2, 1, 3).reshape(BS, O)
         for c in range(N_CORES)], axis=0)
    return out



# revision 31
# speedup vs baseline: 1.1810x; 1.1810x over previous
"""Trainium2 Bass kernel for nn_ExpandingLinear.

Reference computation (B=8192, F0=2048, E1=E2=256, O=1024, F2=2560):
    h1 = concat([x, relu(x[:, e1_parent] * e1_w)], 1)          # [B, 2304]
    h2 = concat([h1, relu(h1[:, e2_parent] * e2_w)], 1)        # [B, 2560]
    W  = scatter_add(zeros(O, F2), (w_rows, w_cols), w_vals)
    b  = scatter_add(zeros(O,), b_idx, b_vals)
    out = h2 @ W.T + b                                          # [B, O]

Algebraic reduction done on the host (weights only):
    relu(x * w) == w * relu(sign(w) * x) for scalar w, so every embed output
    column is (nonneg scalar) * relu(s * x[:, c]) for some source column c and
    sign s.  Folding each embed column's contribution through W gives

        out = x @ W0t + relu(S ⊙ xg) @ A + 1·bias

    where W0t = W[:, :2048].T, xg = the <=511 distinct (c, s) source columns,
    A is a small host-folded matrix, and the all-ones lhsT row adds the bias.

Device kernel (SPMD over 8 cores, batch-sharded 1024 rows/core):
    - DMA x_shard.T (16 k-tiles, scalar queue) + folded weights in n-halves
      ((16+RT) k-tiles, sync queue) + gathered sign-relu columns (RT tiles)
    - memset-sourced PE warm-up bridges the framework preamble to first data
    - ACT: rt = relu(sign * xg); last row is the all-ones bias row
    - PE: out[m,n] accumulates over k-tiles in 4 waves x 2 PSUM half-passes
      (half 0 k-major in DMA arrival order, half 1 group-major so DVE bank
      drains keep ahead of bank-recycling start-matmuls)
    - DVE: PSUM -> SBUF accumulate; last wave stores in 256-col chunks
"""

import numpy as np

import concourse.bass as bass
import concourse.tile as tile
from concourse import bacc, mybir
from concourse.bass_utils import run_bass_kernel_spmd

B, F0, E1, E2, O = 8192, 2048, 256, 256, 1024
F1 = F0 + E1
F2 = F1 + E2
N_CORES = 8
BS = B // N_CORES          # 1024 batch rows per core
P = 128                    # partitions
KT_X = F0 // P             # 16 k-tiles of raw x
N_HALF = 512               # matmul moving free dim (fp32 max)

# matmul operand dtype:
#   float32  — exact, but the PE runs fp32 at 4 cycles/row (~296 us)
#   float32r — TF32 datapath, 1 cycle/row, rel err ~3e-4 (~97 us)
#   bfloat16 — 1 cycle/row + half the DMA bytes, rel err ~2e-3  <== default
MATMUL_DT = mybir.dt.bfloat16

_CACHE = {}


def _fold_weights(e1_w, e2_w, w_vals, b_vals, e1_parent, e2_parent,
                  w_rows, w_cols, b_idx):
    """Host-side weight preprocessing: densify W/b and fold the two embed
    layers' contributions into (cols, signs, A) so the device computes
    out = x @ W0t + relu(sign*x[:, cols]) @ A + bias."""
    W = np.bincount(w_rows.astype(np.int64) * F2 + w_cols.astype(np.int64),
                    weights=w_vals.astype(np.float64),
                    minlength=O * F2).reshape(O, F2)
    bias = np.bincount(b_idx.astype(np.int64), weights=b_vals.astype(np.float64),
                       minlength=O)
    W0t = W[:, :F0].T          # [2048, 1024]
    W1t = W[:, F0:F1].T        # [256, 1024]  layer-1 embed rows
    W2t = W[:, F1:F2].T        # [256, 1024]  layer-2 embed rows

    # each embed column j contributes scale*relu(s*x[:, c]) with weight row w
    # accumulate per (c, s): A_map[(c, s)] += scale * w_row
    A_map = {}

    def acc(c, s, scale, wrow):
        if scale == 0.0:
            return
        key = (int(c), int(s))
        if key in A_map:
            A_map[key] = A_map[key] + scale * wrow
        else:
            A_map[key] = scale * wrow

    e1_parent = e1_parent.astype(np.int64)
    e2_parent = e2_parent.astype(np.int64)
    e1_w64 = e1_w.astype(np.float64)
    e2_w64 = e2_w.astype(np.float64)

    for j in range(E1):
        w = e1_w64[j]
        s = 1 if w >= 0 else -1
        acc(e1_parent[j], s, abs(w), W1t[j])
    for j in range(E2):
        q = e2_parent[j]
        w = e2_w64[j]
        if q < F0:
            s = 1 if w >= 0 else -1
            acc(q, s, abs(w), W2t[j])
        else:
            # refers to layer-1 embed column m1: h1e[:, m1] >= 0 always
            if w < 0:
                continue  # relu(negative * nonneg) == 0
            m1 = q - F0
            w1 = e1_w64[m1]
            s = 1 if w1 >= 0 else -1
            acc(e1_parent[m1], s, w * abs(w1), W2t[j])

    pairs = sorted(A_map.keys())
    n_pairs = len(pairs)
    # relu-block k-tiles; last row of the block is reserved for the bias row
    RT = max(1, -(-(n_pairs + 1) // P))
    n_rows = RT * P
    cols = np.zeros(n_rows, dtype=np.int64)
    signs = np.ones(n_rows, dtype=np.float32)
    A = np.zeros((n_rows, O), dtype=np.float64)
    for i, (c, s) in enumerate(pairs):
        cols[i] = c
        signs[i] = s
        A[i] = A_map[(c, s)]
    return (W0t.astype(np.float32), A.astype(np.float32),
            bias.astype(np.float32), cols, signs, RT)


def _build_program(RT):
    """Build + compile the SPMD Bass program (same for every core)."""
    KT = KT_X + RT  # total k-tiles
    MDT = MATMUL_DT
    nc = bacc.Bacc("TRN2", target_bir_lowering=False, debug=False,
                   num_devices=N_CORES)

    # fp32r relu sources stay f32 (the DVE write rounds); bf16 arrives bf16
    GDT = MDT if MDT == mybir.dt.bfloat16 else mybir.dt.float32
    xt_d = nc.dram_tensor("xt", [KT_X, P, BS], MDT, kind="ExternalInput")
    xg_d = nc.dram_tensor("xg", [RT, P, BS], GDT, kind="ExternalInput")
    wc_d = nc.dram_tensor("wc", [KT, P, O], MDT, kind="ExternalInput")
    sg_d = nc.dram_tensor("sg", [P, RT], mybir.dt.float32,
                          kind="ExternalInput")
    # [m, n, p, c] layout: each [128, 512] half-store is contiguous
    out_d = nc.dram_tensor("out", [BS // P, O // N_HALF, P, N_HALF],
                           mybir.dt.float32, kind="ExternalOutput")

    with tile.TileContext(nc) as tc:
        with (
            tc.tile_pool(name="sbuf", bufs=1) as pool,
            tc.tile_pool(name="outp", bufs=1) as outp,
            tc.tile_pool(name="psum", bufs=8, space="PSUM") as psum,
        ):
            sg_sb = pool.tile([P, RT], mybir.dt.float32, tag="sg")
            nc.gpsimd.dma_start(sg_sb[:], sg_d[:])

            # PE warm-up: data-independent matmuls on a memset constant tile
            # (no DMA dependency, so they start the moment the framework
            # preamble ends) flip the HAM clock gate to 2.4 GHz before the
            # real stream starts (cold matmuls run at 1.2 GHz)
            cst = pool.tile([P, 256], MDT, tag="cst", name="cst")
            nc.vector.memset(cst[:], 1.0)
            wps = psum.tile([P, N_HALF], mybir.dt.float32, tag="ps",
                            name="wps")
            # 14 warm-ups end ~10.7us, bridging to k0's observed arrival
            # (10.2-11.8us): a PE-idle gap >~1us at the warmup->stream
            # transition resets the HAM busy window and postpones the
            # 2.4 GHz unthrottle by another 3.4us
            for _ in range(14):
                nc.tensor.matmul(wps[:, :256], cst[:, :P], cst[:],
                                 start=True, stop=True)

            # lhsT k-tiles (16 raw x + RT sign-relu) and weight k-tiles
            lh = [pool.tile([P, BS], MDT, tag=f"x{kt}", name=f"x{kt}")
                  for kt in range(KT_X)]
            wc = [pool.tile([P, O], MDT, tag=f"w{kt}", name=f"w{kt}")
                  for kt in range(KT)]
            # first k-tile arrives in half chunks so the PE can start on
            # (m0, n0) after ~256 KiB instead of ~512 KiB
            H = BS // 2
            H2 = O // 2
            g_sbs = [pool.tile([P, BS], GDT, tag="g", name=f"g{t}",
                               bufs=RT) for t in range(RT)]
            # wc tiles go on sync in n-halves, lh tiles on scalar: two
            # issue queues, each with a 4-deep flow-control window that
            # paces a single queue to ~165 GB/s -- the head of the kernel
            # is delivery-bound, and 128 KiB wc halves advance the sync
            # window ~2x faster than full tiles.  xg rides after k3 (it
            # feeds the relu whose output isn't consumed until wave 4).
            for kt in range(KT):
                if kt == 0:
                    nc.sync.dma_start(wc[kt][:, :H2], wc_d[kt][:, :H2])
                    nc.scalar.dma_start(lh[kt][:, :H], xt_d[kt][:, :H])
                    nc.sync.dma_start(wc[kt][:, H2:], wc_d[kt][:, H2:])
                    nc.scalar.dma_start(lh[kt][:, H:], xt_d[kt][:, H:])
                else:
                    nc.sync.dma_start(wc[kt][:, :H2], wc_d[kt][:, :H2])
                    nc.sync.dma_start(wc[kt][:, H2:], wc_d[kt][:, H2:])
                    if kt < KT_X:
                        nc.scalar.dma_start(lh[kt][:], xt_d[kt])
                if kt == 3:
                    for t in range(RT):
                        eng = nc.sync if t < RT // 2 else nc.scalar
                        eng.dma_start(g_sbs[t][:], xg_d[t])
            # sign-relu of the gathered columns.  Emitted between wave 1
            # and wave 2 so its scheduler priority sits behind the wave-1
            # drains; the r tiles are only consumed by wave 4 (k >= 16).
            def emit_relu():
                for t in range(RT):
                    r_sb = pool.tile([P, BS], MDT, tag=f"r{t}",
                                     name=f"r{t}")
                    # bias row: xg's last row is all-ones with sign +1, so
                    # the sign-relu passes it through unchanged.  Runs on
                    # the ACT engine (out = relu(in * scale)): the DVE's
                    # in-order queue must stay free for PSUM bank drains --
                    # a relu there blocks on the trickling xg transfers and
                    # stalls the PE's bank-recycling matmuls for ~5 us.
                    nc.scalar.activation(r_sb[:], g_sbs[t][:],
                                         mybir.ActivationFunctionType.Relu,
                                         scale=sg_sb[:, t:t + 1])
                    lh.append(r_sb)

            # K-outer waves of KC tiles: PE consumes k-tiles in DMA arrival
            # order and never waits on far-away tiles. 16 (m, n) output
            # groups > 8 PSUM banks, so each wave runs two passes of 8
            # groups (second pass re-reads the same resident k-tiles).
            # Waves accumulate into o_sb via DVE.
            MT = BS // P           # 8 m-tiles
            NT = O // N_HALF       # 2 n-halves
            groups = [(m, n) for m in range(MT) for n in range(NT)]
            o_sbs = [outp.tile([P, O], mybir.dt.float32, tag=f"o{m}",
                               name=f"o{m}") for m in range(MT)]
            # small waves while the k-stream is in flight, one big wave
            # once everything is resident (fewer DVE accumulate ops)
            waves = [(0, 4), (4, 8), (8, 12), (12, KT)]
            for wi, (k0, k1) in enumerate(waves):
                if wi == 1:
                    emit_relu()
                last_wave = wi == len(waves) - 1
                for half in range(2):
                    gsl = groups[half * 8:(half + 1) * 8]
                    pss = {g: psum.tile([P, N_HALF], mybir.dt.float32,
                                        tag="ps", name="ps") for g in gsl}
                    if last_wave or half == 1:
                        # group-major: each group's start-matmul is followed
                        # by its own k-run (~0.86us) before the next group
                        # needs its recycled PSUM bank, so the 0.68us DVE
                        # drain of that bank always finishes in time.  A
                        # k-major second half would issue 8 bank-reusing
                        # start-matmuls 216ns apart against a serialized
                        # drain chain and stall the PE for several us.
                        # (Half 1 re-reads tiles half 0 already used, so
                        # DMA arrival order doesn't matter here.)
                        order = [(kt, g) for g in gsl for kt in range(k0, k1)]
                    else:
                        # k-major: consume k-tiles in DMA arrival order.
                        # n-outer within each k-tile: the n0 half of each
                        # wc tile lands ~0.8us before the n1 half, so the
                        # four n0 matmuls run while n1 is still in flight
                        gs = sorted(gsl, key=lambda g: (g[1], g[0]))
                        order = [(kt, g) for kt in range(k0, k1) for g in gs]
                    for kt, (m, n) in order:
                        nc.tensor.matmul(
                            pss[(m, n)][:],
                            lh[kt][:, m * P:(m + 1) * P],
                            wc[kt][:, n * N_HALF:(n + 1) * N_HALF],
                            start=(kt == k0),
                            stop=(kt == k1 - 1),
                        )
                        if kt != k1 - 1:
                            continue
                        # group complete: drain its PSUM bank immediately so
                        # the next half's matmuls find a free bank without
                        # waiting (DVE accumulate into the persistent o_sb)
                        if not last_wave:
                            osl = o_sbs[m][:, n * N_HALF:(n + 1) * N_HALF]
                            if wi == 0:
                                nc.vector.tensor_copy(osl, pss[(m, n)][:])
                            else:
                                nc.vector.tensor_add(osl, osl, pss[(m, n)][:])
                            continue
                        # last wave: drain + store in 256-col chunks so the
                        # final group's store chain after its last matmul is
                        # drain(0.35us) -> issue -> small transfer instead of
                        # a serialized 0.7us drain + 0.7us full-tile store
                        CH = 256
                        for ci in range(N_HALF // CH):
                            c0 = n * N_HALF + ci * CH
                            osl = o_sbs[m][:, c0:c0 + CH]
                            nc.vector.tensor_add(
                                osl, osl,
                                pss[(m, n)][:, ci * CH:(ci + 1) * CH])
                            # sync/scalar only: both are idle by now and
                            # their dma_start executes ~0.1us faster than
                            # gpsimd's, which matters on the final chain
                            eng = nc.sync if ci == 0 else nc.scalar
                            eng.dma_start(
                                out_d[m][n][:, ci * CH:(ci + 1) * CH], osl)

    nc.compile()
    return nc


def _round_tf32(a):
    """Round-to-nearest-even fp32 -> tf32 (10-bit mantissa), like the PE's
    fp32r datapath expects (low 13 mantissa bits zero)."""
    u = a.astype(np.float32).view(np.uint32)
    rb = (u >> np.uint32(13)) & np.uint32(1)
    u = (u + np.uint32(0x0FFF) + rb) & np.uint32(0xFFFFE000)
    return u.view(np.float32)


def kernel(input, e1_w, e2_w, w_vals, b_vals, e1_parent, e2_parent,
           w_rows, w_cols, b_idx):
    input = np.asarray(input, dtype=np.float32)
    W0t, A, bias, cols, signs, RT = _fold_weights(
        np.asarray(e1_w), np.asarray(e2_w), np.asarray(w_vals),
        np.asarray(b_vals), np.asarray(e1_parent), np.asarray(e2_parent),
        np.asarray(w_rows), np.asarray(w_cols), np.asarray(b_idx))

    KT = KT_X + RT
    # weight slab: [KT*128, O] = [W0t ; A-with-bias-row]
    wc = np.concatenate([W0t, A], axis=0)
    wc[KT * P - 1, :] = bias           # lhsT row is all-ones -> adds bias
    wc = np.ascontiguousarray(wc.reshape(KT, P, O), dtype=np.float32)
    sg = np.ascontiguousarray(signs.reshape(RT, P).T, dtype=np.float32)

    key = (RT, MATMUL_DT)
    if key not in _CACHE:
        _CACHE[key] = _build_program(RT)
    nc = _CACHE[key]

    xg_full = input[:, cols]           # [B, RT*128] gathered source columns
    xg_full[:, RT * P - 1] = 1.0       # all-ones bias column (sign is +1)
    xmm = input
    if MATMUL_DT == mybir.dt.float32r:
        xmm = _round_tf32(input)
        xg_full = _round_tf32(xg_full)  # relu/sign-mult commute with rounding
        wc = _round_tf32(wc)
    elif MATMUL_DT == mybir.dt.bfloat16:
        import ml_dtypes
        bf = np.dtype(ml_dtypes.bfloat16)
        xmm = input.astype(bf)
        xg_full = xg_full.astype(bf)
        wc = wc.astype(bf)
    in_maps = []
    for c in range(N_CORES):
        sl = slice(c * BS, (c + 1) * BS)
        xt_c = np.ascontiguousarray(xmm[sl].T.reshape(KT_X, P, BS))
        xg_c = np.ascontiguousarray(xg_full[sl].T.reshape(RT, P, BS))
        in_maps.append({"xt": xt_c, "xg": xg_c, "wc": wc, "sg": sg})

    res = run_bass_kernel_spmd(nc, in_maps, list(range(N_CORES)))
    out = np.concatenate(
        [res.results[c]["out"].transpose(0, 2, 1, 3).reshape(BS, O)
         for c in range(N_CORES)], axis=0)
    return out

